# revision 1
# baseline (speedup 1.0000x reference)
"""GAT forward (2-layer graph attention, B=4 N=2048 F=128 H=8 D=64 C=32)
as a Bass/Tile SPMD kernel on 8 Trainium2 NeuronCores.

Sharding: core c -> (batch b=c//2, query-row half c%2).  Each core computes
attention for its 1024 query rows over all 2048 keys for all 8 heads
(layer 1) and for the output head (layer 2).  The only cross-core exchange
is a 2-rank AllGather of the layer-2 projections [Wh2|g1|g2] ([1024,34] f32)
within each (2b, 2b+1) pair.

Key layout decision: attention logits are built TRANSPOSED,
e^T[j (keys) = partitions, i (queries) = free], so that
  - the PV matmul needs no operand transposes at all
    (lhsT = Wh[j,d] stationary, rhs = p[j,i] moving, out = h^T[d,i]),
  - softmax row sums come from a ones-column appended to Wh (PSUM row D).

Per 128x1024 logit tile the streaming work is exactly:
  DVE : one scalar_tensor_tensor  u = (mask_bias + f2[j]) + f1[i]
        (mask_bias in {0,-512} pre-baked host-side into bf16 tiles)
  ACT : Lrelu(u) ; Exp(z)
  PE  : 2 float32r matmuls accumulating h^T (with row-sum column)
Softmax division is deferred to the tiny h^T [64,1024] epilogue
(reciprocal via Ln/Exp of the row-sum), fused with ELU
(elu(v) = relu(v) + exp(min(v,0)) - 1, the -1 folded into a rank-1
correction matmul of the next layer's projection).
"""

import numpy as np
import ml_dtypes

import concourse.bass as bass
import concourse.tile as tile
from concourse import mybir
from concourse.bass_utils import run_bass_kernel_spmd

F32 = mybir.dt.float32
F32R = mybir.dt.float32r
BF16 = mybir.dt.bfloat16

B, N, F, H, D, C = 4, 2048, 128, 8, 64, 32
I = N // 2          # query rows per core
JT = N // 128       # key tiles
IC = I // 128       # query-row 128-chunks per core
KT = (H * D) // 128 # hidden-dim 128-chunks
ALPHA = 0.2
BIG = 512.0         # mask bias; exp(lrelu(-BIG+eps)) underflows to 0 in fp32
N_CORES = 8
REPLICA_GROUPS = [[0, 1], [2, 3], [4, 5], [6, 7]]

ADD = mybir.AluOpType.add
MAX = mybir.AluOpType.max
# NOTE: hardware "Lrelu" has a fixed 0.01 slope and ignores alpha;
# "Prelu" honors alpha (verified on HW) — it is the configurable leaky relu.
ACT_LRELU = mybir.ActivationFunctionType.Prelu
ACT_EXP = mybir.ActivationFunctionType.Exp
ACT_LN = mybir.ActivationFunctionType.Ln


def _split_multiwaits(nc):
    """Pinned walrus accepts only one sync-wait per instruction; Tile's exit
    drain (and occasionally others) carries several.  Hoist extras onto
    single-wait Drains on the same engine immediately before the owner."""
    n_fixed = 0
    for fn in nc.m.functions:
        for bb in fn.blocks:
            for name in [i.name for i in bb.instructions]:
                idx = [i.name for i in bb.instructions].index(name)
                inst = bb.instructions[idx]
                si = inst.sync_info
                if si is None or len(si.on_wait) <= 1:
                    continue
                waits = list(si.on_wait)
                for k, w in enumerate(waits[:-1]):
                    nd = mybir.InstDrain(
                        name=f"waitfix-{inst.name}-{k}", ins=[], outs=[])
                    nd.engine = inst.engine
                    nd.sync_info = mybir.SyncInfo(on_wait=[w], on_update=[])
                    nc.register_instruction(nd, overwrite=True)
                    bb.instructions.insert(idx + k, nd)
                inst.sync_info = mybir.SyncInfo(
                    on_wait=waits[-1:], on_update=list(si.on_update))
                n_fixed += 1
    return n_fixed


N_TILES = H * JT + JT        # 128 layer-1 logit tiles + 16 layer-2
N_PAIRS = N_TILES // 2       # lrelu/exp operate on pairs of tiles


def _spread(n, total, exclude=()):
    """n indices spread evenly over range(total) minus exclude."""
    avail = [t for t in range(total) if t not in exclude]
    if n >= len(avail):
        return set(avail)
    if n <= 0:
        return set()
    idx = np.linspace(0, len(avail) - 1, n).round().astype(int)
    return {avail[i] for i in idx}


def build_program(with_collective=True, cfg=None, repeat=1):
    """cfg routing knobs (engine load balancing across ACT/DVE/GPSIMD):
      gps_mask : #tile-PAIRS (of N_PAIRS) run in P_G mode: mask-add as
                 GPSIMD tensor_tensor + f2col via per-sub ACT Prelu bias
      dve_lrelu: #tile-PAIRS (of the rest) whose leaky-relu runs on DVE
      gps_ep   : route the per-head epilogue normalize-mul to GPSIMD
    """
    cfg = dict(cfg or {})
    gm = _spread(cfg.get("gps_mask", 0), N_PAIRS)
    gl = set()
    dl = _spread(cfg.get("dve_lrelu", 0), N_PAIRS, exclude=gm)
    route = (gm, gl, dl, bool(cfg.get("gps_ep", False)))

    nc = bass.Bass("TRN2", target_bir_lowering=False, debug=False,
                   enable_asserts=False, num_devices=N_CORES)

    xt_d = nc.dram_tensor("xt", [F, N], F32, kind="ExternalInput")
    xtl_d = nc.dram_tensor("xtl", [F, I], F32, kind="ExternalInput")
    mb_d = nc.dram_tensor("mb", [JT, 128, I], BF16, kind="ExternalInput")
    wext_d = nc.dram_tensor("wext", [H, F, D + 2], F32, kind="ExternalInput")
    a1rep_d = nc.dram_tensor("a1rep", [H, F, 128], F32, kind="ExternalInput")
    woext_d = nc.dram_tensor("woext", [KT, 128, C + 2], F32, kind="ExternalInput")
    wcorr_d = nc.dram_tensor("wcorr", [1, C + 2], F32, kind="ExternalInput")
    ident_d = nc.dram_tensor("ident", [128, 128], F32, kind="ExternalInput")
    outp_d = nc.dram_tensor("outp", [I, C], F32, kind="ExternalOutput")

    with tile.TileContext(nc) as tc:
        if repeat > 1:
            # timing rig: run the whole body `repeat` times on-device
            def body(iv, unroll=None):
                _build_body(nc, tc, xt_d, xtl_d, mb_d, wext_d, a1rep_d,
                            woext_d, wcorr_d, ident_d, outp_d,
                            with_collective, route)
            with tc.For_i(0, repeat, 1) as iv:
                body(iv)
        else:
            _build_body(nc, tc, xt_d, xtl_d, mb_d, wext_d, a1rep_d, woext_d,
                        wcorr_d, ident_d, outp_d, with_collective, route)
    _split_multiwaits(nc)
    return nc


def _logit_pair(nc, work, workp, pair_idx, route, tiles):
    """Two key-tiles' logits processed as one [128, 2, I] block, then ONE
    exp over the whole 2*I free dim (amortizes the per-op overhead).

    Modes per pair (GPSIMD supports no *Ptr opcodes, so per-partition-scalar
    STTs are DVE-only; GPS pairs instead fold f2col into the ACT Prelu bias):
      P_G (pair in gm): u[k] = mb + f1rep        (GPSIMD tensor_tensor)
                        z[k] = Prelu(u[k]+f2col) (ACT, per-sub bias)
      STD: u[k] = (mb + f2col) + f1rep           (DVE STT)
           z = Prelu(u) paired on ACT, or max(0.2u, u) STT on DVE (dl)
    """
    gm, gl, dl, _ = route
    u = work.tile([128, 2, I], F32, tag="u")
    if pair_idx in gm:
        for k, (t, mb_ap, f2col_ap, f1rep_ap) in enumerate(tiles):
            nc.gpsimd.tensor_add(u[:, k, :], mb_ap, f1rep_ap)
        for k, (t, mb_ap, f2col_ap, f1rep_ap) in enumerate(tiles):
            nc.scalar.activation(u[:, k, :], u[:, k, :], ACT_LRELU,
                                 bias=f2col_ap, alpha=ALPHA)
    else:
        for k, (t, mb_ap, f2col_ap, f1rep_ap) in enumerate(tiles):
            nc.vector.scalar_tensor_tensor(
                out=u[:, k, :], in0=mb_ap, scalar=f2col_ap, in1=f1rep_ap,
                op0=ADD, op1=ADD)
        if pair_idx in gl or pair_idx in dl:
            nc.vector.scalar_tensor_tensor(
                out=u[:], in0=u[:], scalar=ALPHA, in1=u[:],
                op0=mybir.AluOpType.mult, op1=MAX)
        else:
            nc.scalar.activation(u[:], u[:], ACT_LRELU, alpha=ALPHA)
    p = workp.tile([128, 2, I], F32R, tag="p")
    nc.scalar.activation(p[:], u[:], ACT_EXP)
    return p


def _build_body(nc, tc, xt_d, xtl_d, mb_d, wext_d, a1rep_d, woext_d,
                wcorr_d, ident_d, outp_d, with_collective, route):
    from contextlib import ExitStack
    gps_ep = route[3]
    ctx = ExitStack()
    with ctx:
        singles = ctx.enter_context(tc.tile_pool(name="singles", bufs=1))
        psA = ctx.enter_context(tc.tile_pool(name="psA", bufs=2, space="PSUM"))
        psB = ctx.enter_context(tc.tile_pool(name="psB", bufs=1, space="PSUM"))
        psC = ctx.enter_context(tc.tile_pool(name="psC", bufs=2, space="PSUM"))
        dram = ctx.enter_context(tc.tile_pool(name="dram", bufs=1, space="DRAM"))

        # ---------------- persistent loads ----------------
        # order = head-0 critical path first: the first logit pair needs
        # mb[0:2], f1rep (xtl+a1rep) and fcol[h0] (xt+wext, in phase 0)
        mb_s = singles.tile([128, JT, I], BF16)
        for jt in range(2):
            nc.sync.dma_start(out=mb_s[:, jt, :], in_=mb_d.ap()[jt])
        xtl_s = singles.tile([F, I], F32)
        nc.sync.dma_start(out=xtl_s[:], in_=xtl_d.ap())
        a1rep_s = singles.tile([F, H, 128], F32)
        nc.sync.dma_start(out=a1rep_s[:], in_=a1rep_d.ap().rearrange("h f e -> f h e"))
        for jt in range(2, JT):
            nc.sync.dma_start(out=mb_s[:, jt, :], in_=mb_d.ap()[jt])
        wcorr_s = singles.tile([1, C + 2], F32)
        nc.sync.dma_start(out=wcorr_s[:], in_=wcorr_d.ap())
        ident_s = singles.tile([128, 128], F32)
        nc.sync.dma_start(out=ident_s[:], in_=ident_d.ap())
        woext_s = singles.tile([128, KT, C + 2], F32R)

        ones_s = singles.tile([1, 128], F32)
        nc.gpsimd.memset(ones_s[:], 1.0)

        whbuf = singles.tile([128, H, JT, D + 1], F32R)
        nc.gpsimd.memset(whbuf[:, :, :, D:D + 1].bitcast(F32), 1.0)
        fcol = singles.tile([128, H, JT], F32)
        hcatT = singles.tile([128, KT, I], F32R)

        # ---------------- phase 0 (scoped; freed before work pools) ------
        # Wh tiles + f columns for ALL heads up front; only xt/wext live here.
        with tc.tile_pool(name="ph0", bufs=1) as ph0:
            xt_s = ph0.tile([F, N], F32)
            nc.sync.dma_start(out=xt_s[:], in_=xt_d.ap())
            wext_s = ph0.tile([F, H, D + 2], F32)
            nc.sync.dma_start(out=wext_s[:],
                              in_=wext_d.ap().rearrange("h f e -> f h e"))
            woext_raw = ph0.tile([128, KT, C + 2], F32)
            nc.sync.dma_start(out=woext_raw[:],
                              in_=woext_d.ap().rearrange("k f e -> f k e"))
            nc.any.tensor_copy(out=woext_s[:], in_=woext_raw[:])
            for h in range(H):
                for jt in range(JT):
                    whp = psA.tile([128, D + 2], F32, tag="small")
                    nc.tensor.matmul(whp[:],
                                     lhsT=xt_s[:, jt * 128:(jt + 1) * 128],
                                     rhs=wext_s[:, h, :])
                    nc.any.tensor_copy(out=whbuf[:, h, jt, 0:D],
                                       in_=whp[:, 0:D])
                    nc.any.tensor_copy(out=fcol[:, h, jt:jt + 1],
                                       in_=whp[:, D + 1:D + 2])

        work = ctx.enter_context(tc.tile_pool(name="work", bufs=4))
        workp = ctx.enter_context(tc.tile_pool(name="workp", bufs=3))
        ep1 = ctx.enter_context(tc.tile_pool(name="ep1", bufs=1))
        ep2 = ctx.enter_context(tc.tile_pool(name="ep2", bufs=2))

        # ---------------- layer 1 ----------------
        for h in range(H):
            # f1rep[p, i] = f1[i] (replicated over partitions):
            # lhsT = a1rep[h] (Wa1 in every column), rhs = xT local columns
            f1p = psB.tile([128, I], F32, tag="rep")
            for hf in range(I // 512):
                sl = slice(hf * 512, (hf + 1) * 512)
                nc.tensor.matmul(f1p[:, sl], lhsT=a1rep_s[:, h, :],
                                 rhs=xtl_s[:, sl])
            f1rep_s = ep2.tile([128, I], F32, tag="f1rep")
            nc.any.tensor_copy(out=f1rep_s[:], in_=f1p[:])

            # attention over e^T tiles [j=128, i=I]
            hT = psC.tile([D + 1, I], F32, tag="acc")
            for jp in range(JT // 2):
                tiles = []
                for k in range(2):
                    jt = jp * 2 + k
                    tiles.append((h * JT + jt, mb_s[:, jt, :],
                                  fcol[:, h, jt:jt + 1], f1rep_s[:]))
                p = _logit_pair(nc, work, workp, h * (JT // 2) + jp, route, tiles)
                for k in range(2):
                    jt = jp * 2 + k
                    for hf in range(I // 512):
                        sl = slice(hf * 512, (hf + 1) * 512)
                        nc.tensor.matmul(hT[:, sl],
                                         lhsT=whbuf[:, h, jt, :],
                                         rhs=p[:, k, sl],
                                         start=(jt == 0), stop=(jt == JT - 1))

            # epilogue: r = 1/S via Ln+Exp, v = hT*r, hcat_raw = elu(v)+1
            rh = ep1.tile([1, I], F32, tag="rh")
            nc.scalar.activation(rh[:], hT[D:D + 1, :], ACT_LN)
            nc.scalar.activation(rh[:], rh[:], ACT_EXP, scale=-1.0)
            hT_s = ep2.tile([D, I], F32, tag="hT")
            nc.any.tensor_copy(out=hT_s[:], in_=hT[0:D, :])
            rbcp = psC.tile([D, I], F32, tag="acc")
            for hf in range(I // 512):
                sl = slice(hf * 512, (hf + 1) * 512)
                nc.tensor.matmul(rbcp[:, sl], lhsT=ones_s[0:1, 0:D],
                                 rhs=rh[0:1, sl])
            v = ep1.tile([D, I], F32, tag="v")
            if gps_ep:
                rbc_s = ep1.tile([D, I], F32, tag="rbc")
                nc.any.tensor_copy(out=rbc_s[:], in_=rbcp[:])
                nc.gpsimd.tensor_mul(v[:], hT_s[:], rbc_s[:])
            else:
                nc.vector.tensor_mul(v[:], hT_s[:], rbcp[:])
            t = ep1.tile([D, I], F32, tag="t")
            nc.vector.tensor_scalar_min(t[:], v[:], 0.0)
            nc.scalar.activation(t[:], t[:], ACT_EXP)
            dst = hcatT[(h % 2) * D:(h % 2) * D + D, h // 2, :]
            nc.vector.scalar_tensor_tensor(
                out=dst, in0=v[:], scalar=0.0, in1=t[:], op0=MAX, op1=ADD)

        # ---------------- layer 2 projection + gather ----------------
        wh2loc = singles.tile([128, IC, C + 2], F32)
        gin = dram.tile([I, C + 2], F32)
        for ic in range(IC):
            w2p = psA.tile([128, C + 2], F32, tag="small")
            for kt in range(KT):
                nc.tensor.matmul(
                    w2p[:],
                    lhsT=hcatT[:, kt, ic * 128:(ic + 1) * 128],
                    rhs=woext_s[:, kt, :],
                    start=(kt == 0), stop=False)
            nc.tensor.matmul(w2p[:], lhsT=ones_s[0:1, :], rhs=wcorr_s[:],
                             start=False, stop=True)
            nc.any.tensor_copy(out=wh2loc[:, ic, :], in_=w2p[:])
            nc.sync.dma_start(out=gin[ic * 128:(ic + 1) * 128, :],
                              in_=wh2loc[:, ic, :])

        gout = dram.tile([N, C + 2], F32)
        if with_collective:
            nc.gpsimd.collective_compute(
                "AllGather", mybir.AluOpType.bypass,
                replica_groups=REPLICA_GROUPS,
                ins=[gin.opt()], outs=[gout.opt()])
        else:  # timing-model variant: fake the exchange with two local copies
            nc.sync.dma_start(out=gout[0:I, :], in_=gin[:])
            nc.sync.dma_start(out=gout[I:N, :], in_=gin[:])

        # g1rep[p,i] = g1[i]: transpose local g1 columns into a row, broadcast
        g1rowp = psB.tile([1, I], F32, tag="rep")
        for ic in range(IC):
            nc.tensor.transpose(g1rowp[0:1, ic * 128:(ic + 1) * 128],
                                in_=wh2loc[:, ic, 0:1], identity=ident_s[:])
        g1row_s = ep1.tile([1, I], F32, tag="g1row")
        nc.any.tensor_copy(out=g1row_s[:], in_=g1rowp[:])
        g1rp = psB.tile([128, I], F32, tag="rep")
        for hf in range(I // 512):
            sl = slice(hf * 512, (hf + 1) * 512)
            nc.tensor.matmul(g1rp[:, sl], lhsT=ones_s[0:1, :],
                             rhs=g1row_s[0:1, sl])
        g1rep_s = singles.tile([128, I], F32)
        nc.any.tensor_copy(out=g1rep_s[:], in_=g1rp[:])

        # gathered rows: [g1, g2, Wh2(32)] + ones column -> [128, 35] f32r
        # (DMA writes raw f32 bits; f32r here only tags the matmul operand)
        wh2gr = singles.tile([128, JT, C + 3], F32R)
        nc.gpsimd.memset(wh2gr[:, :, C + 2:C + 3].bitcast(F32), 1.0)
        for jt in range(JT):
            nc.sync.dma_start(out=wh2gr[:, jt, 0:C + 2].bitcast(F32),
                              in_=gout[jt * 128:(jt + 1) * 128, :])

        # ---------------- layer 2 attention ----------------
        o2T = psC.tile([C + 1, I], F32, tag="acc")
        for jp in range(JT // 2):
            tiles = []
            for k in range(2):
                jt = jp * 2 + k
                tiles.append((H * JT + jt, mb_s[:, jt, :],
                              wh2gr[:, jt, 1:2].bitcast(F32), g1rep_s[:]))
            p = _logit_pair(nc, work, workp, H * (JT // 2) + jp, route, tiles)
            for k in range(2):
                jt = jp * 2 + k
                for hf in range(I // 512):
                    sl = slice(hf * 512, (hf + 1) * 512)
                    nc.tensor.matmul(o2T[:, sl],
                                     lhsT=wh2gr[:, jt, 2:C + 3],
                                     rhs=p[:, k, sl],
                                     start=(jt == 0), stop=(jt == JT - 1))

        # ---------------- finalize ----------------
        r2 = ep1.tile([1, I], F32, tag="r2")
        nc.scalar.activation(r2[:], o2T[C:C + 1, :], ACT_LN)
        nc.scalar.activation(r2[:], r2[:], ACT_EXP, scale=-1.0)
        rbc2p = psC.tile([C, I], F32, tag="acc")
        for hf in range(I // 512):
            sl = slice(hf * 512, (hf + 1) * 512)
            nc.tensor.matmul(rbc2p[:, sl], lhsT=ones_s[0:1, 0:C],
                             rhs=r2[0:1, sl])
        rbc2_s = ep1.tile([C, I], F32, tag="rbc2")
        nc.any.tensor_copy(out=rbc2_s[:], in_=rbc2p[:])
        oT_s = ep1.tile([C, I], F32, tag="oT")
        nc.vector.tensor_mul(oT_s[:], o2T[0:C, :], rbc2_s[:])
        for k in range(IC):
            ofp = psA.tile([128, C], F32, tag="small")
            nc.tensor.transpose(ofp[:], in_=oT_s[:, k * 128:(k + 1) * 128],
                                identity=ident_s[0:C, 0:C])
            ofs = ep2.tile([128, C], F32, tag="ofs")
            nc.any.tensor_copy(out=ofs[:], in_=ofp[:])
            nc.sync.dma_start(out=outp_d.ap()[k * 128:(k + 1) * 128, :],
                              in_=ofs[:])


# --------------------------------------------------------------------------
# host side
# --------------------------------------------------------------------------

def shard_inputs(x, adj, W, a1, a2, Wo, ao1, ao2):
    x = np.asarray(x, np.float32)
    adj = np.asarray(adj)
    W = np.asarray(W, np.float32)
    a1 = np.asarray(a1, np.float32)
    a2 = np.asarray(a2, np.float32)
    Wo = np.asarray(Wo, np.float32)
    ao1 = np.asarray(ao1, np.float32)
    ao2 = np.asarray(ao2, np.float32)

    wvec1 = np.einsum("hfd,hd->hf", W, a1)          # [H, F]
    wvec2 = np.einsum("hfd,hd->hf", W, a2)
    wext = np.concatenate([W, wvec1[:, :, None], wvec2[:, :, None]],
                          axis=2).astype(np.float32)
    a1rep = np.repeat(wvec1[:, :, None], 128, axis=2).astype(np.float32)
    wo1 = Wo @ ao1                                   # [512]
    wo2 = Wo @ ao2
    woflat = np.concatenate([wo1[:, None], wo2[:, None], Wo], 1)  # [512, 34]
    woext = woflat.reshape(KT, 128, C + 2).astype(np.float32)
    wcorr = (-woflat.sum(0))[None, :].astype(np.float32)
    ident = np.eye(128, dtype=np.float32)

    in_maps = []
    for c in range(N_CORES):
        b, half = c // 2, c % 2
        i0 = half * I
        xt = np.ascontiguousarray(x[b].T)            # [F, N]
        xtl = np.ascontiguousarray(xt[:, i0:i0 + I])
        adjt = adj[b, i0:i0 + I, :].T                # [N, I] = (j, i)
        mb = np.where(adjt > 0, np.float32(0.0), np.float32(-BIG))
        mb = np.ascontiguousarray(mb.reshape(JT, 128, I)).astype(
            ml_dtypes.bfloat16)
        in_maps.append({
            "xt": xt, "xtl": xtl, "mb": mb, "wext": wext,
            "a1rep": a1rep, "woext": woext, "wcorr": wcorr, "ident": ident,
        })
    return in_maps


# Engine routing chosen by cost-model sweep (TimelineSim) and verified on
# hardware: ACT/DVE/GPSIMD busy ~260/238/124 us, modeled e2e ~366 us/core.
DEFAULT_CFG = {"gps_mask": 28, "dve_lrelu": 38, "gps_ep": True}

_CACHE = {}


def _program():
    if "nc" not in _CACHE:
        _CACHE["nc"] = build_program(with_collective=True, cfg=DEFAULT_CFG)
    return _CACHE["nc"]


def kernel(**inputs):
    nc = _program()
    in_maps = shard_inputs(**inputs)
    res = run_bass_kernel_spmd(nc, in_maps, list(range(N_CORES)))
    _CACHE["last_results"] = res
    out = np.empty((B, N, C), np.float32)
    for c in range(N_CORES):
        b, half = c // 2, c % 2
        out[b, half * I:(half + 1) * I, :] = res.results[c]["outp"]
    return out



# revision 9
# speedup vs baseline: 1.3943x; 1.3943x over previous
"""GAT forward (2-layer graph attention, B=4 N=2048 F=128 H=8 D=64 C=32)
as a Bass/Tile SPMD kernel on 8 Trainium2 NeuronCores.

Sharding: core c -> (batch b=c//2, query-row half c%2).  Each core computes
attention for its 1024 query rows over all 2048 keys for all 8 heads
(layer 1) and for the output head (layer 2).  The only cross-core exchange
is a 2-rank AllGather of the layer-2 projections [g1|g2|Wh2] ([1024,34] f32)
within each (2b, 2b+1) pair.

Key algebraic restructure (vs. the classic mask-add/lrelu/exp pipeline):
  exp(lrelu(u)) = max(exp(u), exp(0.2u))        (exp is monotonic)
and u = f1[i] + f2[j] is rank-1, so exp(u) factorizes into outer products
of vector exps.  Softmax over j is invariant to any per-query factor, so
dividing through by exp(0.2 f1[i]) gives unnormalized weights
  p~[j,i] = max( A[j] * B8[i], A2[j] ) * m[j,i]
with A = exp(f2 - c), A2 = exp(0.2 f2 - c), B8 = exp(0.8 f1), m in {0,1}
the adjacency mask, and c = 4 a global shift keeping fp16 in range.

Per 128x1024 logit tile the N^2 streaming work is exactly TWO ops:
  DVE : q = (B8rep * A[j]) max A2[j]   -- one fused two-scalar
        tensor_scalar in fp16 (4x perf mode, ~0.33us)
  DVE/GPSIMD : p = q * m               -- fp16 tensor_tensor multiply
        (2x mode on DVE, quad-batched; a cfg-chosen subset of quads runs
        on GPSIMD to balance engine load)
  PE  : 2 fp16 matmuls accumulating h^T (with a ones-column row-sum)
All exps are O(N) vector ops on ACT (B8rep per head, A/A2 columns).
Softmax division is deferred to the tiny h^T [64,1024] epilogue
(reciprocal via Ln/Exp of the row-sum), fused with ELU; the ELU's
relu/exp halves are stored separately (hcatT/hcatT2) so the layer-2
projection's PSUM accumulation performs the add, and the -1 is folded
into a rank-1 correction matmul (wcorr).
"""

import numpy as np

import concourse.bass as bass
import concourse.tile as tile
from concourse import mybir
from concourse.bass_utils import run_bass_kernel_spmd

F32 = mybir.dt.float32
F16 = mybir.dt.float16

B, N, F, H, D, C = 4, 2048, 128, 8, 64, 32
I = N // 2          # query rows per core
JT = N // 128       # key tiles
IC = I // 128       # query-row 128-chunks per core
KT = (H * D) // 128 # hidden-dim 128-chunks
ALPHA = 0.2
SHIFT = 4.0         # global logit shift (cancels in softmax); fp16 headroom
N_CORES = 8
REPLICA_GROUPS = [[0, 1], [2, 3], [4, 5], [6, 7]]

MAX = mybir.AluOpType.max
MULT = mybir.AluOpType.mult
ACT_EXP = mybir.ActivationFunctionType.Exp
ACT_LN = mybir.ActivationFunctionType.Ln

QT = JT // 4                 # mult quads per head
N_QUADS = H * QT + QT        # 32 layer-1 quads + 4 layer-2


def _split_multiwaits(nc):
    """Pinned walrus accepts only one sync-wait per instruction; Tile's exit
    drain (and occasionally others) carries several.  Hoist extras onto
    single-wait Drains on the same engine immediately before the owner."""
    n_fixed = 0
    for fn in nc.m.functions:
        for bb in fn.blocks:
            for name in [i.name for i in bb.instructions]:
                idx = [i.name for i in bb.instructions].index(name)
                inst = bb.instructions[idx]
                si = inst.sync_info
                if si is None or len(si.on_wait) <= 1:
                    continue
                waits = list(si.on_wait)
                for k, w in enumerate(waits[:-1]):
                    nd = mybir.InstDrain(
                        name=f"waitfix-{inst.name}-{k}", ins=[], outs=[])
                    nd.engine = inst.engine
                    nd.sync_info = mybir.SyncInfo(on_wait=[w], on_update=[])
                    nc.register_instruction(nd, overwrite=True)
                    bb.instructions.insert(idx + k, nd)
                inst.sync_info = mybir.SyncInfo(
                    on_wait=waits[-1:], on_update=list(si.on_update))
                n_fixed += 1
    return n_fixed


def _spread(n, total, exclude=()):
    """n indices spread evenly over range(total) minus exclude."""
    avail = [t for t in range(total) if t not in exclude]
    if n >= len(avail):
        return set(avail)
    if n <= 0:
        return set()
    idx = np.linspace(0, len(avail) - 1, n).round().astype(int)
    return {avail[i] for i in idx}


def build_program(with_collective=True, cfg=None, repeat=1):
    """cfg knobs:
      gps_quads : #mult-quads (of N_QUADS) whose mask-multiply runs on
                  GPSIMD instead of DVE (engine load balancing)
    """
    cfg = dict(cfg or {})
    gps_set = _spread(cfg.get("gps_quads", 14), N_QUADS)

    nc = bass.Bass("TRN2", target_bir_lowering=False, debug=False,
                   enable_asserts=False, num_devices=N_CORES)

    xt_d = nc.dram_tensor("xt", [F, N], F16, kind="ExternalInput")
    xtl_d = nc.dram_tensor("xtl", [F, I], F16, kind="ExternalInput")
    m_d = nc.dram_tensor("m", [JT, 128, I], F16, kind="ExternalInput")
    wext_d = nc.dram_tensor("wext", [H, F, D + 2], F16, kind="ExternalInput")
    a1rep_d = nc.dram_tensor("a1rep", [H, F, 128], F16, kind="ExternalInput")
    woext_d = nc.dram_tensor("woext", [KT, 128, C + 2], F16, kind="ExternalInput")
    wcorr_d = nc.dram_tensor("wcorr", [1, C + 2], F16, kind="ExternalInput")
    ident_d = nc.dram_tensor("ident", [128, 128], F32, kind="ExternalInput")
    outp_d = nc.dram_tensor("outp", [I, C], F32, kind="ExternalOutput")

    with tile.TileContext(nc) as tc:
        if repeat > 1:
            def body(iv, unroll=None):
                _build_body(nc, tc, xt_d, xtl_d, m_d, wext_d, a1rep_d,
                            woext_d, wcorr_d, ident_d, outp_d,
                            with_collective, gps_set)
            with tc.For_i(0, repeat, 1) as iv:
                body(iv)
        else:
            _build_body(nc, tc, xt_d, xtl_d, m_d, wext_d, a1rep_d, woext_d,
                        wcorr_d, ident_d, outp_d, with_collective, gps_set)
    _split_multiwaits(nc)
    return nc


def _attend(nc, work, workp, psC, m_s, b8, acol, a2col, lhs_of_jt,
            out_pq, quad_base, gps_set):
    """One attention block: for JT key tiles, q = (b8*A) max A2, p = q*m,
    accumulate out_pq[:, sl] += lhsT(jt)^T @ p.  Returns the PSUM tile."""
    for qt in range(QT):
        q = work.tile([128, 4, I], F16, tag="q")
        for k in range(4):
            jt = qt * 4 + k
            nc.vector.tensor_scalar(
                out=q[:, k, :], in0=b8[:],
                scalar1=acol[:, jt:jt + 1], scalar2=a2col[:, jt:jt + 1],
                op0=MULT, op1=MAX)
        p = workp.tile([128, 4, I], F16, tag="p")
        if quad_base + qt in gps_set:
            nc.gpsimd.tensor_mul(p[:], q[:], m_s[:, qt * 4:qt * 4 + 4, :])
        else:
            nc.vector.tensor_mul(p[:], q[:], m_s[:, qt * 4:qt * 4 + 4, :])
        for k in range(4):
            jt = qt * 4 + k
            for hf in range(I // 512):
                sl = slice(hf * 512, (hf + 1) * 512)
                nc.tensor.matmul(out_pq[:, sl],
                                 lhsT=lhs_of_jt(jt),
                                 rhs=p[:, k, sl],
                                 start=(jt == 0), stop=(jt == JT - 1))


def _build_body(nc, tc, xt_d, xtl_d, m_d, wext_d, a1rep_d, woext_d,
                wcorr_d, ident_d, outp_d, with_collective, gps_set):
    from contextlib import ExitStack
    ctx = ExitStack()
    with ctx:
        singles = ctx.enter_context(tc.tile_pool(name="singles", bufs=1))
        psA = ctx.enter_context(tc.tile_pool(name="psA", bufs=2, space="PSUM"))
        psB = ctx.enter_context(tc.tile_pool(name="psB", bufs=1, space="PSUM"))
        psC = ctx.enter_context(tc.tile_pool(name="psC", bufs=2, space="PSUM"))
        dram = ctx.enter_context(tc.tile_pool(name="dram", bufs=1, space="DRAM"))

        # ---------------- persistent loads ----------------
        # head-0 critical path first: quad 0 needs m[0:4], B8rep (xtl+a1rep)
        # and A-columns for h0 (xt+wext in phase 0)
        m_s = singles.tile([128, JT, I], F16)
        for jt in range(4):
            nc.sync.dma_start(out=m_s[:, jt, :], in_=m_d.ap()[jt])
        xtl_s = singles.tile([F, I], F16)
        nc.sync.dma_start(out=xtl_s[:], in_=xtl_d.ap())
        a1rep_s = singles.tile([F, H, 128], F16)
        nc.sync.dma_start(out=a1rep_s[:], in_=a1rep_d.ap().rearrange("h f e -> f h e"))
        for jt in range(4, JT):
            nc.sync.dma_start(out=m_s[:, jt, :], in_=m_d.ap()[jt])
        wcorr_s = singles.tile([1, C + 2], F16)
        nc.sync.dma_start(out=wcorr_s[:], in_=wcorr_d.ap())
        ident_s = singles.tile([128, 128], F32)
        nc.sync.dma_start(out=ident_s[:], in_=ident_d.ap())
        woext_s = singles.tile([128, KT, C + 2], F16)
        nc.sync.dma_start(out=woext_s[:],
                          in_=woext_d.ap().rearrange("k f e -> f k e"))

        ones_s = singles.tile([1, 128], F16)
        nc.gpsimd.memset(ones_s[:], 1.0)
        onesf_s = singles.tile([1, 128], F32)
        nc.gpsimd.memset(onesf_s[:], 1.0)
        negc_s = singles.tile([128, 1], F32)
        nc.gpsimd.memset(negc_s[:], -SHIFT)

        whbuf = singles.tile([128, H, JT, D + 1], F16)
        nc.gpsimd.memset(whbuf[:, :, :, D:D + 1], 1.0)
        fcol = singles.tile([128, H, JT], F32)
        acol = singles.tile([128, H, JT], F32)
        a2col = singles.tile([128, H, JT], F32)
        hcatT = singles.tile([128, KT, I], F16)   # relu half of elu
        hcatT2 = singles.tile([128, KT, I], F16)  # exp half of elu

        # ---------------- phase 0 (scoped; freed before work pools) ------
        # Wh tiles + f2 columns + A/A2 columns for ALL heads up front.
        with tc.tile_pool(name="ph0", bufs=1) as ph0:
            xt_s = ph0.tile([F, N], F16)
            nc.sync.dma_start(out=xt_s[:], in_=xt_d.ap())
            wext_s = ph0.tile([F, H, D + 2], F16)
            nc.sync.dma_start(out=wext_s[:],
                              in_=wext_d.ap().rearrange("h f e -> f h e"))
            for h in range(H):
                for q4 in range(JT // 4):
                    whp = psA.tile([128, 4, D + 2], F32, tag="small")
                    for k in range(4):
                        jt = q4 * 4 + k
                        nc.tensor.matmul(whp[:, k, :],
                                         lhsT=xt_s[:, jt * 128:(jt + 1) * 128],
                                         rhs=wext_s[:, h, :])
                    nc.scalar.copy(out=whbuf[:, h, q4 * 4:q4 * 4 + 4, 0:D],
                                   in_=whp[:, :, 0:D])
                    nc.vector.tensor_copy(out=fcol[:, h, q4 * 4:q4 * 4 + 4],
                                          in_=whp[:, :, D + 1])
                # A = exp(f2 - c), A2 = exp(0.2 f2 - c) for this head
                nc.scalar.activation(acol[:, h, :], fcol[:, h, :], ACT_EXP,
                                     bias=negc_s[:], scale=1.0)
                nc.scalar.activation(a2col[:, h, :], fcol[:, h, :], ACT_EXP,
                                     bias=negc_s[:], scale=ALPHA)

        work = ctx.enter_context(tc.tile_pool(name="work", bufs=3))
        workp = ctx.enter_context(tc.tile_pool(name="workp", bufs=3))
        ep1 = ctx.enter_context(tc.tile_pool(name="ep1", bufs=1))
        ep2 = ctx.enter_context(tc.tile_pool(name="ep2", bufs=2))

        # ---------------- layer 1 ----------------
        for h in range(H):
            # f1 replicated over partitions via all-ones-column trick
            f1p = psB.tile([128, I], F32, tag="rep")
            for hf in range(I // 512):
                sl = slice(hf * 512, (hf + 1) * 512)
                nc.tensor.matmul(f1p[:, sl], lhsT=a1rep_s[:, h, :],
                                 rhs=xtl_s[:, sl])
            b8 = ep2.tile([128, I], F16, tag="b8")
            nc.scalar.activation(b8[:], f1p[:], ACT_EXP, scale=1.0 - ALPHA)

            hT = psC.tile([D + 1, I], F32, tag="acc")
            _attend(nc, work, workp, psC, m_s, b8,
                    acol[:, h, :], a2col[:, h, :],
                    lambda jt: whbuf[:, h, jt, :],
                    hT, h * QT, gps_set)

            # epilogue: r = 1/S via Ln+Exp, v = hT*r, elu split into halves
            hT_s = ep2.tile([D, I], F16, tag="hT")
            nc.scalar.copy(out=hT_s[:], in_=hT[0:D, :])
            rh = ep1.tile([1, I], F32, tag="rh")
            nc.scalar.activation(rh[:], hT[D:D + 1, :], ACT_LN)
            nc.scalar.activation(rh[:], rh[:], ACT_EXP, scale=-1.0)
            rbcp = psC.tile([D, I], F32, tag="acc")
            for hf in range(I // 512):
                sl = slice(hf * 512, (hf + 1) * 512)
                nc.tensor.matmul(rbcp[:, sl], lhsT=onesf_s[0:1, 0:D],
                                 rhs=rh[0:1, sl])
            rbc_s = ep1.tile([D, I], F16, tag="rbc")
            nc.scalar.copy(out=rbc_s[:], in_=rbcp[:])
            v = ep1.tile([D, I], F16, tag="v")
            nc.vector.tensor_mul(v[:], hT_s[:], rbc_s[:])
            t = ep1.tile([D, I], F16, tag="t")
            nc.vector.tensor_scalar_min(t[:], v[:], 0.0)
            dst_r = hcatT[(h % 2) * D:(h % 2) * D + D, h // 2, :]
            nc.vector.tensor_scalar_max(dst_r, v[:], 0.0)
            dst_e = hcatT2[(h % 2) * D:(h % 2) * D + D, h // 2, :]
            nc.scalar.activation(dst_e, t[:], ACT_EXP)

        # ---------------- layer 2 projection + gather ----------------
        wh2loc = singles.tile([128, IC, C + 2], F32)
        gin = dram.tile([I, C + 2], F32)
        for ic in range(IC):
            w2p = psA.tile([128, C + 2], F32, tag="small")
            for kt in range(KT):
                nc.tensor.matmul(
                    w2p[:],
                    lhsT=hcatT[:, kt, ic * 128:(ic + 1) * 128],
                    rhs=woext_s[:, kt, :],
                    start=(kt == 0), stop=False)
            for kt in range(KT):
                nc.tensor.matmul(
                    w2p[:],
                    lhsT=hcatT2[:, kt, ic * 128:(ic + 1) * 128],
                    rhs=woext_s[:, kt, :],
                    start=False, stop=False)
            nc.tensor.matmul(w2p[:], lhsT=ones_s[0:1, :], rhs=wcorr_s[:],
                             start=False, stop=True)
            nc.vector.tensor_copy(out=wh2loc[:, ic, :], in_=w2p[:])
            nc.sync.dma_start(out=gin[ic * 128:(ic + 1) * 128, :],
                              in_=wh2loc[:, ic, :])

        gout = dram.tile([N, C + 2], F32)
        if with_collective:
            nc.gpsimd.collective_compute(
                "AllGather", mybir.AluOpType.bypass,
                replica_groups=REPLICA_GROUPS,
                ins=[gin.opt()], outs=[gout.opt()])
        else:  # timing-model variant: fake the exchange with two local copies
            nc.sync.dma_start(out=gout[0:I, :], in_=gin[:])
            nc.sync.dma_start(out=gout[I:N, :], in_=gin[:])

        # g1rep[p,i] = g1[i]: transpose local g1 columns into a row, broadcast
        g1rowp = psB.tile([1, I], F32, tag="rep")
        for ic in range(IC):
            nc.tensor.transpose(g1rowp[0:1, ic * 128:(ic + 1) * 128],
                                in_=wh2loc[:, ic, 0:1], identity=ident_s[:])
        g1row_s = ep1.tile([1, I], F32, tag="g1row")
        nc.vector.tensor_copy(out=g1row_s[:], in_=g1rowp[:])
        g1rp = psB.tile([128, I], F32, tag="rep")
        for hf in range(I // 512):
            sl = slice(hf * 512, (hf + 1) * 512)
            nc.tensor.matmul(g1rp[:, sl], lhsT=onesf_s[0:1, :],
                             rhs=g1row_s[0:1, sl])
        b8l2 = ep2.tile([128, I], F16, tag="b8")
        nc.scalar.activation(b8l2[:], g1rp[:], ACT_EXP, scale=1.0 - ALPHA)

        # gathered rows: [g1, g2, Wh2(32)] f32 -> fp16 + ones column
        wh2raw = singles.tile([128, JT, C + 2], F32)
        for jt in range(JT):
            nc.sync.dma_start(out=wh2raw[:, jt, :],
                              in_=gout[jt * 128:(jt + 1) * 128, :])
        wh2gr = singles.tile([128, JT, C + 3], F16)
        nc.gpsimd.memset(wh2gr[:, :, C + 2:C + 3], 1.0)
        nc.vector.tensor_copy(out=wh2gr[:, :, 0:C + 2], in_=wh2raw[:])
        acol2 = singles.tile([128, JT], F32)
        a2col2 = singles.tile([128, JT], F32)
        nc.scalar.activation(acol2[:], wh2gr[:, :, 1], ACT_EXP,
                             bias=negc_s[:], scale=1.0)
        nc.scalar.activation(a2col2[:], wh2gr[:, :, 1], ACT_EXP,
                             bias=negc_s[:], scale=ALPHA)

        # ---------------- layer 2 attention ----------------
        o2T = psC.tile([C + 1, I], F32, tag="acc")
        _attend(nc, work, workp, psC, m_s, b8l2, acol2, a2col2,
                lambda jt: wh2gr[:, jt, 2:C + 3],
                o2T, H * QT, gps_set)

        # ---------------- finalize ----------------
        r2 = ep1.tile([1, I], F32, tag="rh")
        nc.scalar.activation(r2[:], o2T[C:C + 1, :], ACT_LN)
        nc.scalar.activation(r2[:], r2[:], ACT_EXP, scale=-1.0)
        rbc2p = psC.tile([C, I], F32, tag="acc")
        for hf in range(I // 512):
            sl = slice(hf * 512, (hf + 1) * 512)
            nc.tensor.matmul(rbc2p[:, sl], lhsT=onesf_s[0:1, 0:C],
                             rhs=r2[0:1, sl])
        rbc2_s = ep1.tile([C, I], F32, tag="rbc")
        nc.scalar.copy(out=rbc2_s[:], in_=rbc2p[:])
        oT_s = ep1.tile([C, I], F32, tag="oT")
        nc.vector.tensor_mul(oT_s[:], o2T[0:C, :], rbc2_s[:])
        for k in range(IC):
            ofp = psA.tile([128, C], F32, tag="small")
            nc.tensor.transpose(ofp[:], in_=oT_s[:, k * 128:(k + 1) * 128],
                                identity=ident_s[0:C, 0:C])
            ofs = ep2.tile([128, C], F32, tag="ofs")
            nc.vector.tensor_copy(out=ofs[:], in_=ofp[:])
            nc.sync.dma_start(out=outp_d.ap()[k * 128:(k + 1) * 128, :],
                              in_=ofs[:])


# --------------------------------------------------------------------------
# host side
# --------------------------------------------------------------------------

def shard_inputs(x, adj, W, a1, a2, Wo, ao1, ao2):
    x = np.asarray(x, np.float32)
    adj = np.asarray(adj)
    W = np.asarray(W, np.float32)
    a1 = np.asarray(a1, np.float32)
    a2 = np.asarray(a2, np.float32)
    Wo = np.asarray(Wo, np.float32)
    ao1 = np.asarray(ao1, np.float32)
    ao2 = np.asarray(ao2, np.float32)

    wvec1 = np.einsum("hfd,hd->hf", W, a1)          # [H, F]
    wvec2 = np.einsum("hfd,hd->hf", W, a2)
    wext = np.concatenate([W, wvec1[:, :, None], wvec2[:, :, None]],
                          axis=2).astype(np.float16)
    a1rep = np.repeat(wvec1[:, :, None], 128, axis=2).astype(np.float16)
    wo1 = Wo @ ao1                                   # [512]
    wo2 = Wo @ ao2
    woflat = np.concatenate([wo1[:, None], wo2[:, None], Wo], 1)  # [512, 34]
    woext = woflat.reshape(KT, 128, C + 2).astype(np.float16)
    wcorr = (-woflat.sum(0))[None, :].astype(np.float16)
    ident = np.eye(128, dtype=np.float32)

    in_maps = []
    for c in range(N_CORES):
        b, half = c // 2, c % 2
        i0 = half * I
        xt = np.ascontiguousarray(x[b].T).astype(np.float16)   # [F, N]
        xtl = np.ascontiguousarray(xt[:, i0:i0 + I])
        adjt = adj[b, i0:i0 + I, :].T                # [N, I] = (j, i)
        m = (adjt > 0).astype(np.float16)
        m = np.ascontiguousarray(m.reshape(JT, 128, I))
        in_maps.append({
            "xt": xt, "xtl": xtl, "m": m, "wext": wext,
            "a1rep": a1rep, "woext": woext, "wcorr": wcorr, "ident": ident,
        })
    return in_maps


# Engine routing chosen by cost-model sweep (TimelineSim) and verified on
# hardware.
DEFAULT_CFG = {"gps_quads": 14}

_CACHE = {}


def _program():
    if "nc" not in _CACHE:
        _CACHE["nc"] = build_program(with_collective=True, cfg=DEFAULT_CFG)
    return _CACHE["nc"]


def kernel(**inputs):
    nc = _program()
    in_maps = shard_inputs(**inputs)
    res = run_bass_kernel_spmd(nc, in_maps, list(range(N_CORES)))
    _CACHE["last_results"] = res
    out = np.empty((B, N, C), np.float32)
    for c in range(N_CORES):
        b, half = c // 2, c % 2
        out[b, half * I:(half + 1) * I, :] = res.results[c]["outp"]
    return out


# revision 22
# speedup vs baseline: 1.5480x; 1.1102x over previous
"""GAT forward (2-layer graph attention, B=4 N=2048 F=128 H=8 D=64 C=32)
as a Bass/Tile SPMD kernel on 8 Trainium2 NeuronCores.

Sharding: core c -> (batch b=c//2, query-row half c%2).  Each core computes
attention for its 1024 query rows over all 2048 keys for all 8 heads
(layer 1) and for the output head (layer 2).  The only cross-core exchange
is a 2-rank AllGather of the layer-2 projections [g1|g2|Wh2] ([1024,34] f32)
within each (2b, 2b+1) pair.

Key algebraic restructure (vs. the classic mask-add/lrelu/exp pipeline):
  exp(lrelu(u)) = max(exp(u), exp(0.2u))        (exp is monotonic)
and u = f1[i] + f2[j] is rank-1, so exp(u) factorizes into outer products
of vector exps.  Softmax over j is invariant to any per-query factor, so
dividing through by exp(0.2 f1[i]) gives unnormalized weights
  p~[j,i] = max( A[j] * B8[i], A2[j] ) * m[j,i]
with A = exp(f2 - c), A2 = exp(0.2 f2 - c), B8 = exp(0.8 f1), m in {0,1}
the adjacency mask, and c = 4 a global shift keeping fp16 in range.

Per 128x1024 logit tile the N^2 streaming work is exactly TWO ops:
  DVE : q = (B8rep * A[j]) max A2[j]   -- one fused two-scalar
        tensor_scalar in fp16 (4x perf mode, ~0.33us)
  DVE/GPSIMD : p = q * m               -- fp16 tensor_tensor multiply
        (2x mode on DVE, quad-batched; a cfg-chosen subset of quads runs
        on GPSIMD to balance engine load)
  PE  : 2 fp16 matmuls accumulating h^T (with a ones-column row-sum)
All exps are O(N) vector ops on ACT (B8rep per head, A/A2 columns).
Softmax division is deferred to the tiny h^T [64,1024] epilogue
(reciprocal via Ln/Exp of the row-sum), fused with ELU; the ELU's
relu/exp halves are stored separately (hcatT/hcatT2) so the layer-2
projection's PSUM accumulation performs the add, and the -1 is folded
into a rank-1 correction matmul (wcorr).
"""

import numpy as np

import concourse.bass as bass
import concourse.tile as tile
from concourse import mybir
from concourse.bass_utils import run_bass_kernel_spmd

F32 = mybir.dt.float32
F32R = mybir.dt.float32r
F16 = mybir.dt.float16

B, N, F, H, D, C = 4, 2048, 128, 8, 64, 32
I = N // 2          # query rows per core
JT = N // 128       # key tiles
IC = I // 128       # query-row 128-chunks per core
KT = (H * D) // 128 # hidden-dim 128-chunks
ALPHA = 0.2
SHIFT = 4.0         # global logit shift (cancels in softmax); fp16 headroom
N_CORES = 8
REPLICA_GROUPS = [[0, 1], [2, 3], [4, 5], [6, 7]]

MAX = mybir.AluOpType.max
MULT = mybir.AluOpType.mult
ACT_EXP = mybir.ActivationFunctionType.Exp
ACT_LN = mybir.ActivationFunctionType.Ln
ACT_RELU = mybir.ActivationFunctionType.Relu

QT = JT // 4                 # mult quads per head
N_QUADS = H * QT + QT        # 32 layer-1 quads + 4 layer-2


def _split_multiwaits(nc):
    """Pinned walrus accepts only one sync-wait per instruction; Tile's exit
    drain (and occasionally others) carries several.  Hoist extras onto
    single-wait Drains on the same engine immediately before the owner."""
    n_fixed = 0
    for fn in nc.m.functions:
        for bb in fn.blocks:
            for name in [i.name for i in bb.instructions]:
                idx = [i.name for i in bb.instructions].index(name)
                inst = bb.instructions[idx]
                si = inst.sync_info
                if si is None or len(si.on_wait) <= 1:
                    continue
                waits = list(si.on_wait)
                for k, w in enumerate(waits[:-1]):
                    nd = mybir.InstDrain(
                        name=f"waitfix-{inst.name}-{k}", ins=[], outs=[])
                    nd.engine = inst.engine
                    nd.sync_info = mybir.SyncInfo(on_wait=[w], on_update=[])
                    nc.register_instruction(nd, overwrite=True)
                    bb.instructions.insert(idx + k, nd)
                inst.sync_info = mybir.SyncInfo(
                    on_wait=waits[-1:], on_update=list(si.on_update))
                n_fixed += 1
    return n_fixed


def _spread(n, total, exclude=()):
    """n indices spread evenly over range(total) minus exclude."""
    avail = [t for t in range(total) if t not in exclude]
    if n >= len(avail):
        return set(avail)
    if n <= 0:
        return set()
    idx = np.linspace(0, len(avail) - 1, n).round().astype(int)
    return {avail[i] for i in idx}


def build_program(with_collective=True, cfg=None, repeat=1):
    """cfg knobs:
      gps_quads : #mult-quads (of N_QUADS) whose mask-multiply runs on
                  GPSIMD instead of DVE (engine load balancing)
    """
    cfg = dict(cfg or {})
    # GPS quads only among the 32 layer-1 quads: layer 2 is latency-critical
    # (post-gather tail) and GPSIMD's ~8us op latency would serialize it.
    gps_set = _spread(cfg.get("gps_quads", 14), H * QT)

    nc = bass.Bass("TRN2", target_bir_lowering=False, debug=False,
                   enable_asserts=False, num_devices=N_CORES)

    xt_d = nc.dram_tensor("xt", [F, N], F16, kind="ExternalInput")
    xtl_d = nc.dram_tensor("xtl", [F, I], F16, kind="ExternalInput")
    m_d = nc.dram_tensor("m", [JT, 128, I], F16, kind="ExternalInput")
    wext_d = nc.dram_tensor("wext", [H, F, D + 2], F16, kind="ExternalInput")
    a1rep_d = nc.dram_tensor("a1rep", [H, F, 128], F16, kind="ExternalInput")
    woext_d = nc.dram_tensor("woext", [KT, 128, C + 2], F16, kind="ExternalInput")
    wcorr_d = nc.dram_tensor("wcorr", [1, C + 2], F16, kind="ExternalInput")
    ident_d = nc.dram_tensor("ident", [128, 128], F32, kind="ExternalInput")
    outp_d = nc.dram_tensor("outp", [I, C], F32, kind="ExternalOutput")

    with tile.TileContext(nc) as tc:
        if repeat > 1:
            def body(iv, unroll=None):
                _build_body(nc, tc, xt_d, xtl_d, m_d, wext_d, a1rep_d,
                            woext_d, wcorr_d, ident_d, outp_d,
                            with_collective, gps_set)
            with tc.For_i(0, repeat, 1) as iv:
                body(iv)
        else:
            _build_body(nc, tc, xt_d, xtl_d, m_d, wext_d, a1rep_d, woext_d,
                        wcorr_d, ident_d, outp_d, with_collective, gps_set)
    _split_multiwaits(nc)
    return nc


def _attend(nc, work, workp, ps, m_s, b8, acol, a2col, lhs_of_jt,
            out_pq, quad_base, gps_set):
    """One attention block: for JT key tiles, q = (b8*A) max A2, p = q*m,
    accumulate out_pq[:, sl] += lhsT(jt)^T @ p.  Returns the PSUM tile."""
    for qt in range(QT):
        q = work.tile([128, 4, I], F16, tag="q")
        for k in range(4):
            jt = qt * 4 + k
            nc.vector.tensor_scalar(
                out=q[:, k, :], in0=b8[:],
                scalar1=acol[:, jt:jt + 1], scalar2=a2col[:, jt:jt + 1],
                op0=MULT, op1=MAX)
        p = workp.tile([128, 4, I], F16, tag="p")
        if quad_base + qt in gps_set:
            nc.gpsimd.tensor_mul(p[:], q[:], m_s[:, qt * 4:qt * 4 + 4, :])
        else:
            nc.vector.tensor_mul(p[:], q[:], m_s[:, qt * 4:qt * 4 + 4, :])
        for k in range(4):
            jt = qt * 4 + k
            for hf in range(I // 512):
                sl = slice(hf * 512, (hf + 1) * 512)
                nc.tensor.matmul(out_pq[:, sl],
                                 lhsT=lhs_of_jt(jt),
                                 rhs=p[:, k, sl],
                                 start=(jt == 0), stop=(jt == JT - 1))


def _build_body(nc, tc, xt_d, xtl_d, m_d, wext_d, a1rep_d, woext_d,
                wcorr_d, ident_d, outp_d, with_collective, gps_set):
    from contextlib import ExitStack
    ctx = ExitStack()
    with ctx:
        singles = ctx.enter_context(tc.tile_pool(name="singles", bufs=1))
        # one rotating PSUM pool: every PSUM tile here is <= 4KB/partition
        # (2 banks), 4 bufs = all 8 banks.  Deep rotation decouples heads.
        ps = ctx.enter_context(tc.tile_pool(name="ps", bufs=4, space="PSUM"))
        dram = ctx.enter_context(tc.tile_pool(name="dram", bufs=1, space="DRAM"))

        # ---------------- persistent loads ----------------
        # head-0 critical path first: quad 0 needs m[0:4], B8rep (xtl+a1rep)
        # and A-columns for h0 (xt+wext in phase 0)
        m_s = singles.tile([128, JT, I], F16)
        for jt in range(4):
            nc.sync.dma_start(out=m_s[:, jt, :], in_=m_d.ap()[jt])
        xtl_s = singles.tile([F, I], F16)
        nc.sync.dma_start(out=xtl_s[:], in_=xtl_d.ap())
        a1rep_s = singles.tile([F, H, 128], F16)
        nc.sync.dma_start(out=a1rep_s[:], in_=a1rep_d.ap().rearrange("h f e -> f h e"))
        for jt in range(4, JT):
            nc.sync.dma_start(out=m_s[:, jt, :], in_=m_d.ap()[jt])
        wcorr_s = singles.tile([1, C + 2], F16)
        nc.sync.dma_start(out=wcorr_s[:], in_=wcorr_d.ap())
        ident_s = singles.tile([128, 128], F32)
        nc.sync.dma_start(out=ident_s[:], in_=ident_d.ap())
        woext_s = singles.tile([128, KT, C + 2], F16)
        nc.sync.dma_start(out=woext_s[:],
                          in_=woext_d.ap().rearrange("k f e -> f k e"))

        ones_s = singles.tile([1, 128], F16)
        nc.gpsimd.memset(ones_s[:], 1.0)
        onesf_s = singles.tile([1, 128], F32R)
        nc.gpsimd.memset(onesf_s[:].bitcast(F32), 1.0)
        negc_s = singles.tile([128, 1], F32)
        nc.gpsimd.memset(negc_s[:], -SHIFT)

        whbuf = singles.tile([128, H, JT, D + 1], F16)
        nc.gpsimd.memset(whbuf[:, :, :, D:D + 1], 1.0)
        fcol = singles.tile([128, H, JT], F32)
        acol = singles.tile([128, H, JT], F32)
        a2col = singles.tile([128, H, JT], F32)
        b8all = singles.tile([128, H, I], F16)    # exp(0.8 f1) per head
        hcatT = singles.tile([128, KT, I], F16)   # relu half of elu
        hcatT2 = singles.tile([128, KT, I], F16)  # exp half of elu

        # ---------------- phase 0 (scoped; freed before work pools) ------
        # B8rep for ALL heads (f1 via the all-ones-column broadcast trick),
        # then Wh tiles + f2 columns + A/A2 columns for ALL heads up front.
        with tc.tile_pool(name="ph0", bufs=1) as ph0:
            for h in range(H):
                f1p = ps.tile([128, I], F32, tag="acc")
                for hf in range(I // 512):
                    sl = slice(hf * 512, (hf + 1) * 512)
                    nc.tensor.matmul(f1p[:, sl], lhsT=a1rep_s[:, h, :],
                                     rhs=xtl_s[:, sl])
                nc.scalar.activation(b8all[:, h, :], f1p[:], ACT_EXP,
                                     scale=1.0 - ALPHA)
            xt_s = ph0.tile([F, N], F16)
            nc.sync.dma_start(out=xt_s[:], in_=xt_d.ap())
            wext_s = ph0.tile([F, H, D + 2], F16)
            nc.sync.dma_start(out=wext_s[:],
                              in_=wext_d.ap().rearrange("h f e -> f h e"))
            for h in range(H):
                for q4 in range(JT // 4):
                    whp = ps.tile([128, 4, D + 2], F32, tag="acc")
                    for k in range(4):
                        jt = q4 * 4 + k
                        nc.tensor.matmul(whp[:, k, :],
                                         lhsT=xt_s[:, jt * 128:(jt + 1) * 128],
                                         rhs=wext_s[:, h, :])
                    nc.scalar.copy(out=whbuf[:, h, q4 * 4:q4 * 4 + 4, 0:D],
                                   in_=whp[:, :, 0:D])
                    nc.vector.tensor_copy(out=fcol[:, h, q4 * 4:q4 * 4 + 4],
                                          in_=whp[:, :, D + 1])
                # A = exp(f2 - c), A2 = exp(0.2 f2 - c) for this head
                nc.scalar.activation(acol[:, h, :], fcol[:, h, :], ACT_EXP,
                                     bias=negc_s[:], scale=1.0)
                nc.scalar.activation(a2col[:, h, :], fcol[:, h, :], ACT_EXP,
                                     bias=negc_s[:], scale=ALPHA)

        work = ctx.enter_context(tc.tile_pool(name="work", bufs=4))
        workp = ctx.enter_context(tc.tile_pool(name="workp", bufs=4))
        ep1 = ctx.enter_context(tc.tile_pool(name="ep1", bufs=2))
        ep2 = ctx.enter_context(tc.tile_pool(name="ep2", bufs=2))

        # ---------------- layer 1 ----------------
        for h in range(H):
            hT = ps.tile([D + 1, I], F32, tag="acc")
            _attend(nc, work, workp, ps, m_s, b8all[:, h, :],
                    acol[:, h, :], a2col[:, h, :],
                    lambda jt: whbuf[:, h, jt, :],
                    hT, h * QT, gps_set)

            # epilogue: r = 1/S via Ln+Exp, v = hT*r, elu split into halves
            hT_s = ep2.tile([D, I], F16, tag="hT")
            nc.scalar.copy(out=hT_s[:], in_=hT[0:D, :])
            rh = ep1.tile([1, I], F32R, tag="rh")
            nc.scalar.activation(rh[:].bitcast(F32), hT[D:D + 1, :], ACT_LN)
            nc.scalar.activation(rh[:].bitcast(F32), rh[:].bitcast(F32),
                                 ACT_EXP, scale=-1.0)
            rbcp = ps.tile([D, I], F32, tag="acc")
            for hf in range(I // 512):
                sl = slice(hf * 512, (hf + 1) * 512)
                nc.tensor.matmul(rbcp[:, sl], lhsT=onesf_s[0:1, 0:D],
                                 rhs=rh[0:1, sl])
            rbc_s = ep1.tile([D, I], F16, tag="rbc")
            nc.scalar.copy(out=rbc_s[:], in_=rbcp[:])
            v = ep1.tile([D, I], F16, tag="v")
            nc.vector.tensor_mul(v[:], hT_s[:], rbc_s[:])
            # elu halves, all on ACT: relu(v) and exp(min(v,0)) = exp(-relu(-v))
            dst_r = hcatT[(h % 2) * D:(h % 2) * D + D, h // 2, :]
            nc.scalar.activation(dst_r, v[:], ACT_RELU)
            t = ep1.tile([D, I], F16, tag="t")
            nc.scalar.activation(t[:], v[:], ACT_RELU, scale=-1.0)
            dst_e = hcatT2[(h % 2) * D:(h % 2) * D + D, h // 2, :]
            nc.scalar.activation(dst_e, t[:], ACT_EXP, scale=-1.0)

        # ---------------- layer 2 projection + gather ----------------
        wh2loc = singles.tile([128, IC, C + 2], F32)
        gin = dram.tile([I, C + 2], F32)
        for ic in range(IC):
            w2p = ps.tile([128, C + 2], F32, tag="acc")
            for kt in range(KT):
                nc.tensor.matmul(
                    w2p[:],
                    lhsT=hcatT[:, kt, ic * 128:(ic + 1) * 128],
                    rhs=woext_s[:, kt, :],
                    start=(kt == 0), stop=False)
            for kt in range(KT):
                nc.tensor.matmul(
                    w2p[:],
                    lhsT=hcatT2[:, kt, ic * 128:(ic + 1) * 128],
                    rhs=woext_s[:, kt, :],
                    start=False, stop=False)
            nc.tensor.matmul(w2p[:], lhsT=ones_s[0:1, :], rhs=wcorr_s[:],
                             start=False, stop=True)
            nc.vector.tensor_copy(out=wh2loc[:, ic, :], in_=w2p[:])
            nc.sync.dma_start(out=gin[ic * 128:(ic + 1) * 128, :],
                              in_=wh2loc[:, ic, :])

        gout = dram.tile([N, C + 2], F32)
        if with_collective:
            nc.gpsimd.collective_compute(
                "AllGather", mybir.AluOpType.bypass,
                replica_groups=REPLICA_GROUPS,
                ins=[gin.opt()], outs=[gout.opt()])
        else:  # timing-model variant: fake the exchange with two local copies
            nc.sync.dma_start(out=gout[0:I, :], in_=gin[:])
            nc.sync.dma_start(out=gout[I:N, :], in_=gin[:])

        # g1rep[p,i] = g1[i]: transpose local g1 columns into a row, broadcast
        g1rowp = ps.tile([1, I], F32, tag="acc")
        for ic in range(IC):
            nc.tensor.transpose(g1rowp[0:1, ic * 128:(ic + 1) * 128],
                                in_=wh2loc[:, ic, 0:1], identity=ident_s[:])
        g1row_s = ep1.tile([1, I], F32R, tag="rh")
        nc.vector.tensor_copy(out=g1row_s[:].bitcast(F32), in_=g1rowp[:])
        g1rp = ps.tile([128, I], F32, tag="acc")
        for hf in range(I // 512):
            sl = slice(hf * 512, (hf + 1) * 512)
            nc.tensor.matmul(g1rp[:, sl], lhsT=onesf_s[0:1, :],
                             rhs=g1row_s[0:1, sl])
        b8l2 = ep2.tile([128, I], F16, tag="b8")
        nc.scalar.activation(b8l2[:], g1rp[:], ACT_EXP, scale=1.0 - ALPHA)

        # gathered rows: [g1, g2, Wh2(32)] f32 -> fp16 + ones column
        wh2raw = singles.tile([128, JT, C + 2], F32)
        for jt in range(JT):
            nc.sync.dma_start(out=wh2raw[:, jt, :],
                              in_=gout[jt * 128:(jt + 1) * 128, :])
        wh2gr = singles.tile([128, JT, C + 3], F16)
        nc.gpsimd.memset(wh2gr[:, :, C + 2:C + 3], 1.0)
        nc.vector.tensor_copy(out=wh2gr[:, :, 0:C + 2], in_=wh2raw[:])
        acol2 = singles.tile([128, JT], F32)
        a2col2 = singles.tile([128, JT], F32)
        nc.scalar.activation(acol2[:], wh2gr[:, :, 1], ACT_EXP,
                             bias=negc_s[:], scale=1.0)
        nc.scalar.activation(a2col2[:], wh2gr[:, :, 1], ACT_EXP,
                             bias=negc_s[:], scale=ALPHA)

        # ---------------- layer 2 attention ----------------
        o2T = ps.tile([C + 1, I], F32, tag="acc")
        _attend(nc, work, workp, ps, m_s, b8l2, acol2, a2col2,
                lambda jt: wh2gr[:, jt, 2:C + 3],
                o2T, H * QT, gps_set)

        # ---------------- finalize ----------------
        r2 = ep1.tile([1, I], F32R, tag="rh")
        nc.scalar.activation(r2[:].bitcast(F32), o2T[C:C + 1, :], ACT_LN)
        nc.scalar.activation(r2[:].bitcast(F32), r2[:].bitcast(F32),
                             ACT_EXP, scale=-1.0)
        rbc2p = ps.tile([C, I], F32, tag="acc")
        for hf in range(I // 512):
            sl = slice(hf * 512, (hf + 1) * 512)
            nc.tensor.matmul(rbc2p[:, sl], lhsT=onesf_s[0:1, 0:C],
                             rhs=r2[0:1, sl])
        rbc2_s = ep1.tile([C, I], F32, tag="rbc")
        nc.scalar.copy(out=rbc2_s[:], in_=rbc2p[:])
        oT_s = ep1.tile([C, I], F32, tag="oT")
        nc.vector.tensor_mul(oT_s[:], o2T[0:C, :], rbc2_s[:])
        for k in range(IC):
            ofp = ps.tile([128, C], F32, tag="acc")
            nc.tensor.transpose(ofp[:], in_=oT_s[:, k * 128:(k + 1) * 128],
                                identity=ident_s[0:C, 0:C])
            ofs = ep2.tile([128, C], F32, tag="ofs")
            nc.vector.tensor_copy(out=ofs[:], in_=ofp[:])
            nc.sync.dma_start(out=outp_d.ap()[k * 128:(k + 1) * 128, :],
                              in_=ofs[:])


# --------------------------------------------------------------------------
# host side
# --------------------------------------------------------------------------

def shard_inputs(x, adj, W, a1, a2, Wo, ao1, ao2):
    x = np.asarray(x, np.float32)
    adj = np.asarray(adj)
    W = np.asarray(W, np.float32)
    a1 = np.asarray(a1, np.float32)
    a2 = np.asarray(a2, np.float32)
    Wo = np.asarray(Wo, np.float32)
    ao1 = np.asarray(ao1, np.float32)
    ao2 = np.asarray(ao2, np.float32)

    wvec1 = np.einsum("hfd,hd->hf", W, a1)          # [H, F]
    wvec2 = np.einsum("hfd,hd->hf", W, a2)
    wext = np.concatenate([W, wvec1[:, :, None], wvec2[:, :, None]],
                          axis=2).astype(np.float16)
    a1rep = np.repeat(wvec1[:, :, None], 128, axis=2).astype(np.float16)
    wo1 = Wo @ ao1                                   # [512]
    wo2 = Wo @ ao2
    woflat = np.concatenate([wo1[:, None], wo2[:, None], Wo], 1)  # [512, 34]
    woext = woflat.reshape(KT, 128, C + 2).astype(np.float16)
    wcorr = (-woflat.sum(0))[None, :].astype(np.float16)
    ident = np.eye(128, dtype=np.float32)

    in_maps = []
    for c in range(N_CORES):
        b, half = c // 2, c % 2
        i0 = half * I
        xt = np.ascontiguousarray(x[b].T).astype(np.float16)   # [F, N]
        xtl = np.ascontiguousarray(xt[:, i0:i0 + I])
        adjt = adj[b, i0:i0 + I, :].T                # [N, I] = (j, i)
        m = (adjt > 0).astype(np.float16)
        m = np.ascontiguousarray(m.reshape(JT, 128, I))
        in_maps.append({
            "xt": xt, "xtl": xtl, "m": m, "wext": wext,
            "a1rep": a1rep, "woext": woext, "wcorr": wcorr, "ident": ident,
        })
    return in_maps


# Engine routing chosen by cost-model sweep (TimelineSim) and verified on
# hardware.
DEFAULT_CFG = {"gps_quads": 14}

_CACHE = {}


def _program():
    if "nc" not in _CACHE:
        _CACHE["nc"] = build_program(with_collective=True, cfg=DEFAULT_CFG)
    return _CACHE["nc"]


def kernel(**inputs):
    nc = _program()
    in_maps = shard_inputs(**inputs)
    res = run_bass_kernel_spmd(nc, in_maps, list(range(N_CORES)))
    _CACHE["last_results"] = res
    out = np.empty((B, N, C), np.float32)
    for c in range(N_CORES):
        b, half = c // 2, c % 2
        out[b, half * I:(half + 1) * I, :] = res.results[c]["outp"]
    return out
